# revision 10
# baseline (speedup 1.0000x reference)
"""LoRA linear kernel for Trainium2 (8 NeuronCores, SPMD data-parallel).

Computes y = x @ (B @ A)^T for
    x: [4, 2048, 4096] f32, B: [4096, 16] f32, A: [16, 4096] f32.

Strategy: never materialize W = B @ A.  Factor as t = x @ A^T (rank 16)
then y = t @ B^T.  Tokens (4*2048 = 8192) are sharded across 8 cores
(1024 tokens each); A and B are replicated.  bf16 on the wire both
ways (rel err ~5e-3 vs the 2e-2 gate).

v6 (trace-driven; v1 63.0, v2 63.9, v3 66.8, v4 64.5, v5 78.5 us):
  - graded window = [first framework memset, last teardown inst];
    ~8.7us teardown is fixed; the lever is the LAST y write landing.
  - HAM is the governing constraint: the PE clock halves (K=4/8) when
    any ~3.4us activity window is insufficiently busy, and once
    dropped it has been observed STUCK at K=4 for 10s of us even under
    100% PE load (v5 tail).  At K=4 a [*,512] matmul is 610ns > the
    ~335ns/bank PSUM-evacuation pace, so the PE becomes the write-
    production binder and the write phase balloons to ~30us.  v6
    therefore keeps PE density >=~90% in EVERY window from first to
    last matmul: junk matmuls are interleaved where no real work can
    fill, and they are CHEAP (16-wide stationary so LDWEIGHTS ~92ns
    hides, free dim 256 -> ~115ns each; v5's 128-wide junks cost
    380-420ns and overloaded the PE).
  - PSUM evacuation [128,512] f32->bf16 is ~658ns on DVE / ~687ns on
    ACT (PSUM has 1 read port, fp32 = 1x mode; GPSIMD has no PSUM
    port).  Split 4:4 (v1's 6:2 made DVE a 4us/chunk chain) and issue
    the y DMA from the Sync queue so the ACT queue only does evacs.
  - Rings: x chunks + y writes on the SP ring (one ring sustains
    ~425GB/s = the per-NC cap; multi-queue gives no aggregate gain),
    consts on the ACT ring (in front of the x reads they cost ~3us).
  - 2 groups x 512 tokens (G=4 spread production but starved PE
    density); mm2(g0,c) interleaves octets(g1); mm2(g0,3) + all of
    mm2(g1) are octet-less so they carry the junk fill.
"""

import sys

import numpy as np

if "/opt/trn_rl_repo" not in sys.path:
    sys.path.insert(0, "/opt/trn_rl_repo")

# Problem shape (hardcoded per contract)
BATCH = 4
SEQ = 2048
D = 4096          # in_features == out_features
R = 16            # lora rank
NCORES = 8
NTOK = BATCH * SEQ            # 8192 tokens total
TOK = NTOK // NCORES          # 1024 tokens per core
P = 128                       # partitions
KO = D // P                   # 32 feature chunks
TB = 512                      # tokens per mm1 group (matmul free dim)
NG = TOK // TB                # 2 groups per core
NCHG = 4                      # x DMA chunks per group (1MB each)
KOC = KO // NCHG              # 8 ko-slices per chunk
NB = 512                      # matmul free dim for mm2 (psum bank limit)

# Module-level knobs for test.py (harness never touches these)
TRACE = False
LAST_RESULTS = None

_nc_cache = None


def _build_program():
    from concourse import bacc, mybir, tile

    nc = bacc.Bacc(
        "TRN2", target_bir_lowering=False, debug=False, num_devices=NCORES
    )

    f32 = mybir.dt.float32
    bf16 = mybir.dt.bfloat16

    xt = nc.dram_tensor("xt", [NG, NCHG, P, KOC, TB], bf16, kind="ExternalInput")
    at = nc.dram_tensor("at", [P, KO, R], bf16, kind="ExternalInput")
    bt = nc.dram_tensor("bt", [R, D], bf16, kind="ExternalInput")
    y = nc.dram_tensor("y", [TOK, D], bf16, kind="ExternalOutput")

    with tile.TileContext(nc) as tc:
        with (
            tc.tile_pool(name="consts", bufs=1) as consts,
            tc.tile_pool(name="xin", bufs=NG * NCHG) as xin,
            tc.tile_pool(name="tbuf", bufs=2) as tbuf,
            tc.tile_pool(name="yout", bufs=8) as yout,
            tc.tile_pool(name="pt", bufs=1, space="PSUM") as pt_pool,
            tc.tile_pool(name="py", bufs=3, space="PSUM") as py_pool,
            tc.tile_pool(name="warmp", bufs=1, space="PSUM") as warm_pool,
        ):
            # consts on the ACT ring; the SP ring head stays free for x.
            at_s = consts.tile([P, KO, R], bf16)
            nc.scalar.dma_start(at_s[:], at[:])
            bt_s = consts.tile([R, D], bf16)
            nc.scalar.dma_start(bt_s[:], bt[:])

            junk = consts.tile([P, NB], bf16)
            nc.gpsimd.memset(junk[:], 0.0)
            warm_t = warm_pool.tile([P, NB], f32)

            def pe_junk(n, free=256):
                # cheap PE filler: 16-wide stationary (LDWEIGHTS hides),
                # ~115ns each at K=8.
                for _ in range(n):
                    nc.tensor.matmul(
                        warm_t[:R, :free], junk[:, :R], junk[:, :free],
                        start=True, stop=True, skip_group_check=True,
                    )

            def pe_warm(n):
                # prologue ramp junk: big enough to register activity
                for _ in range(n):
                    nc.tensor.matmul(
                        warm_t[:], junk[:, :P], junk[:],
                        start=True, stop=True, skip_group_check=True,
                    )

            pe_warm(8)
            tc.no_sync_barrier()

            # Prefetch ALL x chunks first: the Sync queue must issue
            # every x read before the first y write, or the y write's
            # evac-wait stalls the queue and starves later reads (v6).
            xts = {}
            for g in range(NG):
                for c4 in range(NCHG):
                    t_ = xin.tile([P, KOC, TB], bf16, tag="xt")
                    nc.sync.dma_start(t_[:], xt[g, c4])
                    xts[(g, c4)] = t_

            def mm1_octet(g, c4, psum_t):
                # one 1MB fully-contiguous x chunk -> 8 accumulating matmuls
                xt_tile = xts[(g, c4)]
                for j in range(KOC):
                    ko = c4 * KOC + j
                    nc.tensor.matmul(
                        psum_t[:],
                        at_s[:, ko, :],
                        xt_tile[:, j, :],
                        start=(ko == 0),
                        stop=(ko == KO - 1),
                        skip_group_check=True,
                    )

            def make_tT(psum_t):
                tT = tbuf.tile([R, TB], bf16)
                nc.vector.tensor_copy(tT[:], psum_t[:])
                return tT

            def mm2_chunk(g, c, tT, fill=0, split_write=False):
                y_row = yout.tile([P, D], bf16)
                for pair in range(D // (2 * NB)):
                    # two matmuls into one 2-bank psum tile, then ONE
                    # 2-bank evacuation (FD=1024): ~600ns/bank on DVE
                    # vs 658 single, and half the op count.
                    psum_y = py_pool.tile([P, 2 * NB], f32, tag="psum_y")
                    for h in range(2):
                        n = 2 * pair + h
                        nc.tensor.matmul(
                            psum_y[:, h * NB : (h + 1) * NB],
                            tT[:, c * P : (c + 1) * P],
                            bt_s[:, n * NB : (n + 1) * NB],
                            start=True,
                            stop=True,
                            skip_group_check=True,
                        )
                        if fill:
                            pe_junk(fill)
                    o0 = 2 * pair * NB
                    if pair % 2 == 0:
                        nc.vector.tensor_copy(y_row[:, o0 : o0 + 2 * NB], psum_y[:])
                    else:
                        nc.scalar.copy(y_row[:, o0 : o0 + 2 * NB], psum_y[:])
                row0 = g * TB + c * P
                # y writes issue from the Sync queue onto the SP ring
                # (FIFO behind the reads; SBUF backlog bridges).
                if split_write:
                    nc.sync.dma_start(y[row0 : row0 + P, : D // 2], y_row[:, : D // 2])
                    nc.sync.dma_start(y[row0 : row0 + P, D // 2 :], y_row[:, D // 2 :])
                else:
                    nc.sync.dma_start(y[row0 : row0 + P, :], y_row[:])

            # ---- software-pipelined schedule (see module docstring) ----
            psum_t0 = pt_pool.tile([R, TB], f32, tag="psum_t")
            for c4 in range(NCHG):
                mm1_octet(0, c4, psum_t0)
                pe_junk(4)          # fill the read-gated octet gap
            tT0 = make_tT(psum_t0)
            pe_junk(4)

            psum_t1 = pt_pool.tile([R, TB], f32, tag="psum_t")
            for c4 in range(NCHG):
                mm1_octet(1, c4, psum_t1)
                if c4 < 3:
                    mm2_chunk(0, c4, tT0)
            tT1 = make_tT(psum_t1)
            mm2_chunk(0, 3, tT0, fill=1)

            for c in range(NCHG):
                mm2_chunk(1, c, tT1, fill=1, split_write=(c == NCHG - 1))

    nc.finalize()
    return nc


def kernel(x, lora_matrix_B, lora_matrix_A):
    global _nc_cache, LAST_RESULTS
    import ml_dtypes
    from concourse.bass_utils import run_bass_kernel_spmd

    if _nc_cache is None:
        _nc_cache = _build_program()
    nc = _nc_cache

    bf16 = ml_dtypes.bfloat16
    x_flat = np.asarray(x, dtype=np.float32).reshape(NTOK, D).astype(bf16)
    A = np.asarray(lora_matrix_A, dtype=np.float32).astype(bf16)
    B = np.asarray(lora_matrix_B, dtype=np.float32).astype(bf16)

    # at[p, ko, j] = A[j, ko*128 + p];  bt[j, o] = B[o, j]
    at_prep = np.ascontiguousarray(A.reshape(R, KO, P).transpose(2, 1, 0))
    bt_prep = np.ascontiguousarray(B.T)

    in_maps = []
    for core in range(NCORES):
        xc = x_flat[core * TOK : (core + 1) * TOK, :]
        # xt[g, c4, p, j, t] = xc[g*512 + t, (c4*8 + j)*128 + p]
        xt_prep = np.ascontiguousarray(
            xc.reshape(NG, TB, NCHG, KOC, P).transpose(0, 2, 4, 3, 1)
        )
        in_maps.append({"xt": xt_prep, "at": at_prep, "bt": bt_prep})

    res = run_bass_kernel_spmd(
        nc, in_maps, core_ids=list(range(NCORES)), trace=TRACE
    )
    LAST_RESULTS = res

    y = np.concatenate([res.results[c]["y"] for c in range(NCORES)], axis=0)
    return y.reshape(BATCH, SEQ, D).astype(np.float32)


# revision 12
# speedup vs baseline: 1.0542x; 1.0542x over previous
"""LoRA linear kernel for Trainium2 (8 NeuronCores, SPMD data-parallel).

Computes y = x @ (B @ A)^T for
    x: [4, 2048, 4096] f32, B: [4096, 16] f32, A: [16, 4096] f32.

Strategy: never materialize W = B @ A.  Factor as t = x @ A^T (rank 16)
then y = t @ B^T.  Tokens (4*2048 = 8192) are sharded across 8 cores
(1024 tokens each); A and B are replicated.  bf16 on the wire both
ways (rel err ~5e-3 vs the 2e-2 gate).

v6 (trace-driven; v1 63.0, v2 63.9, v3 66.8, v4 64.5, v5 78.5 us):
  - graded window = [first framework memset, last teardown inst];
    ~8.7us teardown is fixed; the lever is the LAST y write landing.
  - HAM is the governing constraint: the PE clock halves (K=4/8) when
    any ~3.4us activity window is insufficiently busy, and once
    dropped it has been observed STUCK at K=4 for 10s of us even under
    100% PE load (v5 tail).  At K=4 a [*,512] matmul is 610ns > the
    ~335ns/bank PSUM-evacuation pace, so the PE becomes the write-
    production binder and the write phase balloons to ~30us.  v6
    therefore keeps PE density >=~90% in EVERY window from first to
    last matmul: junk matmuls are interleaved where no real work can
    fill, and they are CHEAP (16-wide stationary so LDWEIGHTS ~92ns
    hides, free dim 256 -> ~115ns each; v5's 128-wide junks cost
    380-420ns and overloaded the PE).
  - PSUM evacuation [128,512] f32->bf16 is ~658ns on DVE / ~687ns on
    ACT (PSUM has 1 read port, fp32 = 1x mode; GPSIMD has no PSUM
    port).  Split 4:4 (v1's 6:2 made DVE a 4us/chunk chain) and issue
    the y DMA from the Sync queue so the ACT queue only does evacs.
  - Rings: x chunks + y writes on the SP ring (one ring sustains
    ~425GB/s = the per-NC cap; multi-queue gives no aggregate gain),
    consts on the ACT ring (in front of the x reads they cost ~3us).
  - 2 groups x 512 tokens (G=4 spread production but starved PE
    density); mm2(g0,c) interleaves octets(g1); mm2(g0,3) + all of
    mm2(g1) are octet-less so they carry the junk fill.
"""

import sys

import numpy as np

if "/opt/trn_rl_repo" not in sys.path:
    sys.path.insert(0, "/opt/trn_rl_repo")

# Problem shape (hardcoded per contract)
BATCH = 4
SEQ = 2048
D = 4096          # in_features == out_features
R = 16            # lora rank
NCORES = 8
NTOK = BATCH * SEQ            # 8192 tokens total
TOK = NTOK // NCORES          # 1024 tokens per core
P = 128                       # partitions
KO = D // P                   # 32 feature chunks
TB = 512                      # tokens per mm1 group (matmul free dim)
NG = TOK // TB                # 2 groups per core
NCHG = 4                      # x DMA chunks per group (1MB each)
KOC = KO // NCHG              # 8 ko-slices per chunk
NB = 512                      # matmul free dim for mm2 (psum bank limit)

# Module-level knobs for test.py (harness never touches these)
TRACE = False
LAST_RESULTS = None

_nc_cache = None


def _build_program():
    from concourse import bacc, mybir, tile

    nc = bacc.Bacc(
        "TRN2", target_bir_lowering=False, debug=False, num_devices=NCORES
    )

    f32 = mybir.dt.float32
    bf16 = mybir.dt.bfloat16

    xt = nc.dram_tensor("xt", [NG, NCHG, P, KOC, TB], bf16, kind="ExternalInput")
    at = nc.dram_tensor("at", [P, KO, R], bf16, kind="ExternalInput")
    bt = nc.dram_tensor("bt", [R, D], bf16, kind="ExternalInput")
    y = nc.dram_tensor("y", [TOK, D], bf16, kind="ExternalOutput")

    with tile.TileContext(nc) as tc:
        with (
            tc.tile_pool(name="consts", bufs=1) as consts,
            tc.tile_pool(name="xin", bufs=NG * NCHG) as xin,
            tc.tile_pool(name="tbuf", bufs=2) as tbuf,
            tc.tile_pool(name="yout", bufs=8) as yout,
            tc.tile_pool(name="pt", bufs=1, space="PSUM") as pt_pool,
            tc.tile_pool(name="py", bufs=6, space="PSUM") as py_pool,
            tc.tile_pool(name="warmp", bufs=1, space="PSUM") as warm_pool,
        ):
            # consts on the ACT ring; the SP ring head stays free for x.
            at_s = consts.tile([P, KO, R], bf16)
            nc.scalar.dma_start(at_s[:], at[:])
            bt_s = consts.tile([R, D], bf16)
            nc.scalar.dma_start(bt_s[:], bt[:])

            junk = consts.tile([P, NB], bf16)
            nc.gpsimd.memset(junk[:], 0.0)
            warm_t = warm_pool.tile([P, NB], f32)

            def pe_junk(n, free=256):
                # cheap PE filler: 16-wide stationary (LDWEIGHTS hides),
                # ~115ns each at K=8.
                for _ in range(n):
                    nc.tensor.matmul(
                        warm_t[:R, :free], junk[:, :R], junk[:, :free],
                        start=True, stop=True, skip_group_check=True,
                    )

            def pe_warm(n):
                # prologue ramp junk: registers HAM activity with a
                # 16-wide stationary (1/8 array energy — the late-run
                # K=4 drop is power-driven, so filler must be cheap).
                for _ in range(n):
                    nc.tensor.matmul(
                        warm_t[:R, :], junk[:, :R], junk[:],
                        start=True, stop=True, skip_group_check=True,
                    )

            pe_warm(8)
            tc.no_sync_barrier()

            # Prefetch ALL x chunks first: the Sync queue must issue
            # every x read before the first y write, or the y write's
            # evac-wait stalls the queue and starves later reads (v6).
            xts = {}
            for g in range(NG):
                for c4 in range(NCHG):
                    t_ = xin.tile([P, KOC, TB], bf16, tag="xt")
                    nc.sync.dma_start(t_[:], xt[g, c4])
                    xts[(g, c4)] = t_

            def mm1_octet(g, c4, psum_t):
                # one 1MB fully-contiguous x chunk -> 8 accumulating matmuls
                xt_tile = xts[(g, c4)]
                for j in range(KOC):
                    ko = c4 * KOC + j
                    nc.tensor.matmul(
                        psum_t[:],
                        at_s[:, ko, :],
                        xt_tile[:, j, :],
                        start=(ko == 0),
                        stop=(ko == KO - 1),
                        skip_group_check=True,
                    )

            def make_tT(psum_t):
                tT = tbuf.tile([R, TB], bf16)
                nc.vector.tensor_copy(tT[:], psum_t[:])
                return tT

            def mm2_chunk(g, c, tT, fill=0):
                y_row = yout.tile([P, D], bf16)
                for n in range(D // NB):
                    psum_y = py_pool.tile([P, NB], f32, tag="psum_y")
                    nc.tensor.matmul(
                        psum_y[:],
                        tT[:, c * P : (c + 1) * P],
                        bt_s[:, n * NB : (n + 1) * NB],
                        start=True,
                        stop=True,
                        skip_group_check=True,
                    )
                    if fill:
                        pe_junk(fill)
                    # Single-bank PSUM evacuation, DVE 4 : ACT 4
                    if n % 2 == 0:
                        nc.vector.tensor_copy(y_row[:, n * NB : (n + 1) * NB], psum_y[:])
                    else:
                        nc.scalar.copy(y_row[:, n * NB : (n + 1) * NB], psum_y[:])
                row0 = g * TB + c * P
                # y writes issue from the Sync queue onto the SP ring
                # (FIFO behind the reads; SBUF backlog bridges).
                nc.sync.dma_start(y[row0 : row0 + P, :], y_row[:])

            # ---- software-pipelined schedule (see module docstring) ----
            psum_t0 = pt_pool.tile([R, TB], f32, tag="psum_t")
            for c4 in range(NCHG):
                mm1_octet(0, c4, psum_t0)
                pe_junk(4)          # fill the read-gated octet gap
            tT0 = make_tT(psum_t0)
            pe_junk(4)

            psum_t1 = pt_pool.tile([R, TB], f32, tag="psum_t")
            for c4 in range(NCHG):
                mm1_octet(1, c4, psum_t1)
                if c4 < 3:
                    mm2_chunk(0, c4, tT0)
            tT1 = make_tT(psum_t1)
            mm2_chunk(0, 3, tT0, fill=1)

            for c in range(NCHG):
                mm2_chunk(1, c, tT1, fill=1)

    nc.finalize()
    return nc


def kernel(x, lora_matrix_B, lora_matrix_A):
    global _nc_cache, LAST_RESULTS
    import ml_dtypes
    from concourse.bass_utils import run_bass_kernel_spmd

    if _nc_cache is None:
        _nc_cache = _build_program()
    nc = _nc_cache

    bf16 = ml_dtypes.bfloat16
    x_flat = np.asarray(x, dtype=np.float32).reshape(NTOK, D).astype(bf16)
    A = np.asarray(lora_matrix_A, dtype=np.float32).astype(bf16)
    B = np.asarray(lora_matrix_B, dtype=np.float32).astype(bf16)

    # at[p, ko, j] = A[j, ko*128 + p];  bt[j, o] = B[o, j]
    at_prep = np.ascontiguousarray(A.reshape(R, KO, P).transpose(2, 1, 0))
    bt_prep = np.ascontiguousarray(B.T)

    in_maps = []
    for core in range(NCORES):
        xc = x_flat[core * TOK : (core + 1) * TOK, :]
        # xt[g, c4, p, j, t] = xc[g*512 + t, (c4*8 + j)*128 + p]
        xt_prep = np.ascontiguousarray(
            xc.reshape(NG, TB, NCHG, KOC, P).transpose(0, 2, 4, 3, 1)
        )
        in_maps.append({"xt": xt_prep, "at": at_prep, "bt": bt_prep})

    res = run_bass_kernel_spmd(
        nc, in_maps, core_ids=list(range(NCORES)), trace=TRACE
    )
    LAST_RESULTS = res

    y = np.concatenate([res.results[c]["y"] for c in range(NCORES)], axis=0)
    return y.reshape(BATCH, SEQ, D).astype(np.float32)


# revision 14
# speedup vs baseline: 1.0575x; 1.0031x over previous
"""LoRA linear kernel for Trainium2 (8 NeuronCores, SPMD data-parallel).

Computes y = x @ (B @ A)^T for
    x: [4, 2048, 4096] f32, B: [4096, 16] f32, A: [16, 4096] f32.

Strategy: never materialize W = B @ A.  Factor as t = x @ A^T (rank 16)
then y = t @ B^T.  Tokens (4*2048 = 8192) are sharded across 8 cores
(1024 tokens each); A and B are replicated.  bf16 on the wire both
ways (rel err ~5e-3 vs the 2e-2 gate).

v6 (trace-driven; v1 63.0, v2 63.9, v3 66.8, v4 64.5, v5 78.5 us):
  - graded window = [first framework memset, last teardown inst];
    ~8.7us teardown is fixed; the lever is the LAST y write landing.
  - HAM is the governing constraint: the PE clock halves (K=4/8) when
    any ~3.4us activity window is insufficiently busy, and once
    dropped it has been observed STUCK at K=4 for 10s of us even under
    100% PE load (v5 tail).  At K=4 a [*,512] matmul is 610ns > the
    ~335ns/bank PSUM-evacuation pace, so the PE becomes the write-
    production binder and the write phase balloons to ~30us.  v6
    therefore keeps PE density >=~90% in EVERY window from first to
    last matmul: junk matmuls are interleaved where no real work can
    fill, and they are CHEAP (16-wide stationary so LDWEIGHTS ~92ns
    hides, free dim 256 -> ~115ns each; v5's 128-wide junks cost
    380-420ns and overloaded the PE).
  - PSUM evacuation [128,512] f32->bf16 is ~658ns on DVE / ~687ns on
    ACT (PSUM has 1 read port, fp32 = 1x mode; GPSIMD has no PSUM
    port).  Split 4:4 (v1's 6:2 made DVE a 4us/chunk chain) and issue
    the y DMA from the Sync queue so the ACT queue only does evacs.
  - Rings: x chunks + y writes on the SP ring (one ring sustains
    ~425GB/s = the per-NC cap; multi-queue gives no aggregate gain),
    consts on the ACT ring (in front of the x reads they cost ~3us).
  - 2 groups x 512 tokens (G=4 spread production but starved PE
    density); mm2(g0,c) interleaves octets(g1); mm2(g0,3) + all of
    mm2(g1) are octet-less so they carry the junk fill.
"""

import sys

import numpy as np

if "/opt/trn_rl_repo" not in sys.path:
    sys.path.insert(0, "/opt/trn_rl_repo")

# Problem shape (hardcoded per contract)
BATCH = 4
SEQ = 2048
D = 4096          # in_features == out_features
R = 16            # lora rank
NCORES = 8
NTOK = BATCH * SEQ            # 8192 tokens total
TOK = NTOK // NCORES          # 1024 tokens per core
P = 128                       # partitions
KO = D // P                   # 32 feature chunks
TB = 512                      # tokens per mm1 group (matmul free dim)
NG = TOK // TB                # 2 groups per core
NCHG = 4                      # x DMA chunks per group (1MB each)
KOC = KO // NCHG              # 8 ko-slices per chunk
NB = 512                      # matmul free dim for mm2 (psum bank limit)

# Module-level knobs for test.py (harness never touches these)
TRACE = False
LAST_RESULTS = None

_nc_cache = None


def _build_program():
    from concourse import bacc, mybir, tile

    nc = bacc.Bacc(
        "TRN2", target_bir_lowering=False, debug=False, num_devices=NCORES
    )

    f32 = mybir.dt.float32
    bf16 = mybir.dt.bfloat16

    xt = nc.dram_tensor("xt", [NG, NCHG, P, KOC, TB], bf16, kind="ExternalInput")
    at = nc.dram_tensor("at", [P, KO, R], bf16, kind="ExternalInput")
    bt = nc.dram_tensor("bt", [R, D], bf16, kind="ExternalInput")
    y = nc.dram_tensor("y", [TOK, D], bf16, kind="ExternalOutput")

    with tile.TileContext(nc) as tc:
        with (
            tc.tile_pool(name="consts", bufs=1) as consts,
            tc.tile_pool(name="xin", bufs=NG * NCHG) as xin,
            tc.tile_pool(name="tbuf", bufs=2) as tbuf,
            tc.tile_pool(name="yout", bufs=8) as yout,
            tc.tile_pool(name="pt", bufs=1, space="PSUM") as pt_pool,
            tc.tile_pool(name="py", bufs=6, space="PSUM") as py_pool,
            tc.tile_pool(name="warmp", bufs=1, space="PSUM") as warm_pool,
        ):
            # consts on the ACT ring; the SP ring head stays free for x.
            at_s = consts.tile([P, KO, R], bf16)
            nc.scalar.dma_start(at_s[:], at[:])
            bt_s = consts.tile([R, D], bf16)
            nc.scalar.dma_start(bt_s[:], bt[:])

            junk = consts.tile([P, NB], bf16)
            nc.gpsimd.memset(junk[:], 0.0)
            warm_t = warm_pool.tile([P, NB], f32)

            def pe_junk(n, free=256):
                # cheap PE filler: 16-wide stationary (LDWEIGHTS hides),
                # ~115ns each at K=8.
                for _ in range(n):
                    nc.tensor.matmul(
                        warm_t[:R, :free], junk[:, :R], junk[:, :free],
                        start=True, stop=True, skip_group_check=True,
                    )

            def pe_warm(n):
                # prologue ramp junk: registers HAM activity with a
                # 16-wide stationary (1/8 array energy — the late-run
                # K=4 drop is power-driven, so filler must be cheap).
                for _ in range(n):
                    nc.tensor.matmul(
                        warm_t[:R, :], junk[:, :R], junk[:],
                        start=True, stop=True, skip_group_check=True,
                    )

            pe_warm(8)
            tc.no_sync_barrier()

            # Prefetch ALL x chunks first: the Sync queue must issue
            # every x read before the first y write, or the y write's
            # evac-wait stalls the queue and starves later reads (v6).
            xts = {}
            for g in range(NG):
                for c4 in range(NCHG):
                    t_ = xin.tile([P, KOC, TB], bf16, tag="xt")
                    nc.sync.dma_start(t_[:], xt[g, c4])
                    xts[(g, c4)] = t_

            def mm1_octet(g, c4, psum_t):
                # one 1MB fully-contiguous x chunk -> 8 accumulating matmuls
                xt_tile = xts[(g, c4)]
                for j in range(KOC):
                    ko = c4 * KOC + j
                    nc.tensor.matmul(
                        psum_t[:],
                        at_s[:, ko, :],
                        xt_tile[:, j, :],
                        start=(ko == 0),
                        stop=(ko == KO - 1),
                        skip_group_check=True,
                    )

            def make_tT(psum_t):
                tT = tbuf.tile([R, TB], bf16)
                nc.vector.tensor_copy(tT[:], psum_t[:])
                return tT

            def mm2_chunk(g, c, tT, fill=0, split_write=1):
                y_row = yout.tile([P, D], bf16)
                row0 = g * TB + c * P
                for n in range(D // NB):
                    psum_y = py_pool.tile([P, NB], f32, tag="psum_y")
                    nc.tensor.matmul(
                        psum_y[:],
                        tT[:, c * P : (c + 1) * P],
                        bt_s[:, n * NB : (n + 1) * NB],
                        start=True,
                        stop=True,
                        skip_group_check=True,
                    )
                    if fill:
                        pe_junk(fill)
                    # Single-bank PSUM evacuation, DVE 4 : ACT 4
                    if n % 2 == 0:
                        nc.vector.tensor_copy(y_row[:, n * NB : (n + 1) * NB], psum_y[:])
                    else:
                        nc.scalar.copy(y_row[:, n * NB : (n + 1) * NB], psum_y[:])
                # y writes issue from the Sync queue onto the SP ring
                # (FIFO behind the reads; SBUF backlog bridges).  The
                # final chunk splits its write so only the last piece's
                # wire latency is exposed on the kernel tail.
                ds = D // split_write
                for s in range(split_write):
                    nc.sync.dma_start(
                        y[row0 : row0 + P, s * ds : (s + 1) * ds],
                        y_row[:, s * ds : (s + 1) * ds],
                    )

            # ---- software-pipelined schedule (see module docstring) ----
            psum_t0 = pt_pool.tile([R, TB], f32, tag="psum_t")
            for c4 in range(NCHG):
                mm1_octet(0, c4, psum_t0)
                pe_junk(4)          # fill the read-gated octet gap
            tT0 = make_tT(psum_t0)
            pe_junk(4)

            psum_t1 = pt_pool.tile([R, TB], f32, tag="psum_t")
            for c4 in range(NCHG):
                mm1_octet(1, c4, psum_t1)
                if c4 < 3:
                    mm2_chunk(0, c4, tT0)
            tT1 = make_tT(psum_t1)
            mm2_chunk(0, 3, tT0, fill=1)

            for c in range(NCHG):
                mm2_chunk(1, c, tT1, fill=1, split_write=(4 if c == NCHG - 1 else 1))

    nc.finalize()
    return nc


def kernel(x, lora_matrix_B, lora_matrix_A):
    global _nc_cache, LAST_RESULTS
    import ml_dtypes
    from concourse.bass_utils import run_bass_kernel_spmd

    if _nc_cache is None:
        _nc_cache = _build_program()
    nc = _nc_cache

    bf16 = ml_dtypes.bfloat16
    x_flat = np.asarray(x, dtype=np.float32).reshape(NTOK, D).astype(bf16)
    A = np.asarray(lora_matrix_A, dtype=np.float32).astype(bf16)
    B = np.asarray(lora_matrix_B, dtype=np.float32).astype(bf16)

    # at[p, ko, j] = A[j, ko*128 + p];  bt[j, o] = B[o, j]
    at_prep = np.ascontiguousarray(A.reshape(R, KO, P).transpose(2, 1, 0))
    bt_prep = np.ascontiguousarray(B.T)

    in_maps = []
    for core in range(NCORES):
        xc = x_flat[core * TOK : (core + 1) * TOK, :]
        # xt[g, c4, p, j, t] = xc[g*512 + t, (c4*8 + j)*128 + p]
        xt_prep = np.ascontiguousarray(
            xc.reshape(NG, TB, NCHG, KOC, P).transpose(0, 2, 4, 3, 1)
        )
        in_maps.append({"xt": xt_prep, "at": at_prep, "bt": bt_prep})

    res = run_bass_kernel_spmd(
        nc, in_maps, core_ids=list(range(NCORES)), trace=TRACE
    )
    LAST_RESULTS = res

    y = np.concatenate([res.results[c]["y"] for c in range(NCORES)], axis=0)
    return y.reshape(BATCH, SEQ, D).astype(np.float32)


# revision 15
# speedup vs baseline: 1.0630x; 1.0053x over previous
"""LoRA linear kernel for Trainium2 (8 NeuronCores, SPMD data-parallel).

Computes y = x @ (B @ A)^T for
    x: [4, 2048, 4096] f32, B: [4096, 16] f32, A: [16, 4096] f32.

Strategy: never materialize W = B @ A.  Factor as t = x @ A^T (rank 16)
then y = t @ B^T.  Tokens (4*2048 = 8192) are sharded across 8 cores
(1024 tokens each); A and B are replicated.  bf16 on the wire both
ways (rel err ~5e-3 vs the 2e-2 gate).

final schedule (trace-driven; v1 baseline 63.0us -> 60.7us):
  - graded window = [first framework memset, last teardown inst];
    ~8.7us teardown (NEFF-wrapper sem resets) is fixed; the lever is
    when the LAST y write lands.
  - HAM/throttle governs everything: the PE clock halves (K=4/8) when
    an ~3.4us activity window is too idle, AND after ~30us of
    sustained PE activity a power/thermal throttle drops K=4 anyway
    (observed at ~41-43us in every run, even with PE ~95% busy; once
    dropped it can stick).  At K=4 a [*,512] matmul issues every
    ~427ns > the ~335ns/bank PSUM-evacuation pace, so the throttled
    PE becomes the write-production binder.  Fillers must therefore
    be cheap in ENERGY, not just time: all junk/warm matmuls use a
    16-wide stationary (1/8 of the PE array).
  - PSUM evacuation [128,512] f32->bf16 is ~658ns on DVE / ~687ns on
    ACT (PSUM has 1 read port, fp32 = 1x mode; GPSIMD has no PSUM
    port; 2-bank FD=1024 evacs measured WORSE).  Split 4:4 (v1's 6:2
    made DVE a 4us/chunk serial chain) and issue the y DMA from the
    Sync queue so the ACT queue only does evacs.
  - Rings: consts on the ACT ring (at the head of the SP ring they
    delay chunk0 ~3us and idle the PE into a HAM drop).  All x chunks
    are prefetched on the SP ring BEFORE any y write is issued from
    the Sync queue: a y-write instruction's evac-wait otherwise
    stalls the queue and starves the remaining x reads (cost ~14us).
    One ring sustains ~425GB/s = the per-NC cap; extra queues just
    round-robin the same SDMA pool (no aggregate gain).
  - 2 groups x 512 tokens (4-group variants starved PE density and
    triggered early HAM drops); mm2(g0,c) runs between octets(g1);
    the octet-less chunks (g0 c3, all g1) interleave one cheap junk
    per matmul to stay HAM-busy at the evacuation pace; the final
    chunk splits its 1MB write 4x so only 256KB of wire latency is
    exposed on the tail.
"""

import sys

import numpy as np

if "/opt/trn_rl_repo" not in sys.path:
    sys.path.insert(0, "/opt/trn_rl_repo")

# Problem shape (hardcoded per contract)
BATCH = 4
SEQ = 2048
D = 4096          # in_features == out_features
R = 16            # lora rank
NCORES = 8
NTOK = BATCH * SEQ            # 8192 tokens total
TOK = NTOK // NCORES          # 1024 tokens per core
P = 128                       # partitions
KO = D // P                   # 32 feature chunks
TB = 512                      # tokens per mm1 group (matmul free dim)
NG = TOK // TB                # 2 groups per core
NCHG = 4                      # x DMA chunks per group (1MB each)
KOC = KO // NCHG              # 8 ko-slices per chunk
NB = 512                      # matmul free dim for mm2 (psum bank limit)

# Module-level knobs for test.py (harness never touches these)
TRACE = False
LAST_RESULTS = None

_nc_cache = None


def _build_program():
    from concourse import bacc, mybir, tile

    nc = bacc.Bacc(
        "TRN2", target_bir_lowering=False, debug=False, num_devices=NCORES
    )

    f32 = mybir.dt.float32
    bf16 = mybir.dt.bfloat16

    xt = nc.dram_tensor("xt", [NG, NCHG, P, KOC, TB], bf16, kind="ExternalInput")
    at = nc.dram_tensor("at", [P, KO, R], bf16, kind="ExternalInput")
    bt = nc.dram_tensor("bt", [R, D], bf16, kind="ExternalInput")
    y = nc.dram_tensor("y", [TOK, D], bf16, kind="ExternalOutput")

    with tile.TileContext(nc) as tc:
        with (
            tc.tile_pool(name="consts", bufs=1) as consts,
            tc.tile_pool(name="xin", bufs=NG * NCHG) as xin,
            tc.tile_pool(name="tbuf", bufs=2) as tbuf,
            tc.tile_pool(name="yout", bufs=8) as yout,
            tc.tile_pool(name="pt", bufs=1, space="PSUM") as pt_pool,
            tc.tile_pool(name="py", bufs=6, space="PSUM") as py_pool,
            tc.tile_pool(name="warmp", bufs=1, space="PSUM") as warm_pool,
        ):
            # consts on the ACT ring; the SP ring head stays free for x.
            at_s = consts.tile([P, KO, R], bf16)
            nc.scalar.dma_start(at_s[:], at[:])
            bt_s = consts.tile([R, D], bf16)
            nc.scalar.dma_start(bt_s[:], bt[:])

            junk = consts.tile([P, NB], bf16)
            nc.gpsimd.memset(junk[:], 0.0)
            warm_t = warm_pool.tile([P, NB], f32)

            def pe_junk(n, free=256):
                # cheap PE filler: 16-wide stationary (LDWEIGHTS hides),
                # ~115ns each at K=8.
                for _ in range(n):
                    nc.tensor.matmul(
                        warm_t[:R, :free], junk[:, :R], junk[:, :free],
                        start=True, stop=True, skip_group_check=True,
                    )

            def pe_warm(n):
                # prologue ramp junk: registers HAM activity with a
                # 16-wide stationary (1/8 array energy — the late-run
                # K=4 drop is power-driven, so filler must be cheap).
                for _ in range(n):
                    nc.tensor.matmul(
                        warm_t[:R, :], junk[:, :R], junk[:],
                        start=True, stop=True, skip_group_check=True,
                    )

            pe_warm(8)
            tc.no_sync_barrier()

            # Prefetch ALL x chunks first: the Sync queue must issue
            # every x read before the first y write, or the y write's
            # evac-wait stalls the queue and starves later reads (v6).
            xts = {}
            for g in range(NG):
                for c4 in range(NCHG):
                    t_ = xin.tile([P, KOC, TB], bf16, tag="xt")
                    nc.sync.dma_start(t_[:], xt[g, c4])
                    xts[(g, c4)] = t_

            def mm1_octet(g, c4, psum_t):
                # one 1MB fully-contiguous x chunk -> 8 accumulating matmuls
                xt_tile = xts[(g, c4)]
                for j in range(KOC):
                    ko = c4 * KOC + j
                    nc.tensor.matmul(
                        psum_t[:],
                        at_s[:, ko, :],
                        xt_tile[:, j, :],
                        start=(ko == 0),
                        stop=(ko == KO - 1),
                        skip_group_check=True,
                    )

            def make_tT(psum_t):
                tT = tbuf.tile([R, TB], bf16)
                nc.vector.tensor_copy(tT[:], psum_t[:])
                return tT

            def mm2_chunk(g, c, tT, fill=0, split_write=1):
                y_row = yout.tile([P, D], bf16)
                row0 = g * TB + c * P
                for n in range(D // NB):
                    psum_y = py_pool.tile([P, NB], f32, tag="psum_y")
                    nc.tensor.matmul(
                        psum_y[:],
                        tT[:, c * P : (c + 1) * P],
                        bt_s[:, n * NB : (n + 1) * NB],
                        start=True,
                        stop=True,
                        skip_group_check=True,
                    )
                    if fill:
                        pe_junk(fill)
                    # Single-bank PSUM evacuation, DVE 4 : ACT 4
                    if n % 2 == 0:
                        nc.vector.tensor_copy(y_row[:, n * NB : (n + 1) * NB], psum_y[:])
                    else:
                        nc.scalar.copy(y_row[:, n * NB : (n + 1) * NB], psum_y[:])
                # y writes issue from the Sync queue onto the SP ring
                # (FIFO behind the reads; SBUF backlog bridges).  The
                # final chunk splits its write so only the last piece's
                # wire latency is exposed on the kernel tail.
                ds = D // split_write
                for s in range(split_write):
                    nc.sync.dma_start(
                        y[row0 : row0 + P, s * ds : (s + 1) * ds],
                        y_row[:, s * ds : (s + 1) * ds],
                    )

            # ---- software-pipelined schedule (see module docstring) ----
            psum_t0 = pt_pool.tile([R, TB], f32, tag="psum_t")
            for c4 in range(NCHG):
                mm1_octet(0, c4, psum_t0)
                pe_junk(4)          # fill the read-gated octet gap
            tT0 = make_tT(psum_t0)
            pe_junk(4)

            psum_t1 = pt_pool.tile([R, TB], f32, tag="psum_t")
            for c4 in range(NCHG):
                mm1_octet(1, c4, psum_t1)
                if c4 < 3:
                    mm2_chunk(0, c4, tT0)
            tT1 = make_tT(psum_t1)
            mm2_chunk(0, 3, tT0, fill=1)

            for c in range(NCHG):
                mm2_chunk(1, c, tT1, fill=1, split_write=(4 if c == NCHG - 1 else 1))

    nc.finalize()
    return nc


def kernel(x, lora_matrix_B, lora_matrix_A):
    global _nc_cache, LAST_RESULTS
    import ml_dtypes
    from concourse.bass_utils import run_bass_kernel_spmd

    if _nc_cache is None:
        _nc_cache = _build_program()
    nc = _nc_cache

    bf16 = ml_dtypes.bfloat16
    x_flat = np.asarray(x, dtype=np.float32).reshape(NTOK, D).astype(bf16)
    A = np.asarray(lora_matrix_A, dtype=np.float32).astype(bf16)
    B = np.asarray(lora_matrix_B, dtype=np.float32).astype(bf16)

    # at[p, ko, j] = A[j, ko*128 + p];  bt[j, o] = B[o, j]
    at_prep = np.ascontiguousarray(A.reshape(R, KO, P).transpose(2, 1, 0))
    bt_prep = np.ascontiguousarray(B.T)

    in_maps = []
    for core in range(NCORES):
        xc = x_flat[core * TOK : (core + 1) * TOK, :]
        # xt[g, c4, p, j, t] = xc[g*512 + t, (c4*8 + j)*128 + p]
        xt_prep = np.ascontiguousarray(
            xc.reshape(NG, TB, NCHG, KOC, P).transpose(0, 2, 4, 3, 1)
        )
        in_maps.append({"xt": xt_prep, "at": at_prep, "bt": bt_prep})

    res = run_bass_kernel_spmd(
        nc, in_maps, core_ids=list(range(NCORES)), trace=TRACE
    )
    LAST_RESULTS = res

    y = np.concatenate([res.results[c]["y"] for c in range(NCORES)], axis=0)
    return y.reshape(BATCH, SEQ, D).astype(np.float32)


# revision 16
# speedup vs baseline: 1.0769x; 1.0130x over previous
"""LoRA linear kernel for Trainium2 (8 NeuronCores, SPMD data-parallel).

Computes y = x @ (B @ A)^T for
    x: [4, 2048, 4096] f32, B: [4096, 16] f32, A: [16, 4096] f32.

Strategy: never materialize W = B @ A.  Factor as t = x @ A^T (rank 16)
then y = t @ B^T.  Tokens (4*2048 = 8192) are sharded across 8 cores
(1024 tokens each); A and B are replicated.  bf16 on the wire both
ways (rel err ~5e-3 vs the 2e-2 gate).

Schedule (trace-driven; v1 baseline 63.0us -> 60.3us -> this):
  - graded window = [first framework memset, last teardown inst];
    ~8.7us teardown (NEFF-wrapper sem resets) is fixed; the lever is
    when the LAST y write lands.
  - HAM/throttle governs everything: the PE clock halves (K=4/8) when
    an ~3.4us activity window is too idle, AND after ~30us of
    sustained PE activity a power/thermal throttle drops K=4 anyway
    (observed ~42.5us, even with PE ~95% busy; once dropped it can
    stick).  At K=4 a [*,512] matmul issues every ~427ns > the
    ~335ns/bank PSUM-evacuation pace, so the throttled PE becomes the
    write-production binder.  Fillers are cheap in ENERGY: 16-wide
    stationary matmuls (~115ns, 1/8 of the array).
  - PSUM evacuation [128,512] f32->bf16 is ~658ns on DVE / ~687ns on
    ACT (PSUM has 1 read port, fp32 = 1x mode; GPSIMD has no PSUM
    port; 2-bank FD=1024 evacs measured WORSE).  Split 4:4 and issue
    the y DMA from the Sync queue so the ACT queue only does evacs.
  - Rings: consts on the ACT ring; ALL x chunks prefetched on the SP
    ring BEFORE any y write is issued from the Sync queue (a y-write's
    evac-wait otherwise stalls the queue and starves later reads).
    One ring sustains ~425GB/s = the per-NC cap; extra queues just
    round-robin the same SDMA pool.
  - ASYMMETRIC GROUPS (256, 256, 512 tokens): the first evacuation in
    the symmetric 512/512 layout waited for all 4MB of group 0
    (first evac ~24.9us).  A 256-token first group completes its
    contraction after 2MB, starting production ~8us earlier and
    overlapping more of the evacuation stream under the read phase.
    mm1 octets at free=256 issue at 109ns/matmul (LDWEIGHTS hides).
  - mm2(group k) chunks interleave the octets of group k+1 at chunk
    granularity; the final group's mm2 (no octets left) interleaves
    one cheap junk per matmul to stay HAM-busy at the evacuation
    pace; the final chunk splits its 1MB write 4x so only 256KB of
    wire latency is exposed on the tail.
"""

import sys

import numpy as np

if "/opt/trn_rl_repo" not in sys.path:
    sys.path.insert(0, "/opt/trn_rl_repo")

# Problem shape (hardcoded per contract)
BATCH = 4
SEQ = 2048
D = 4096          # in_features == out_features
R = 16            # lora rank
NCORES = 8
NTOK = BATCH * SEQ            # 8192 tokens total
TOK = NTOK // NCORES          # 1024 tokens per core
P = 128                       # partitions
KO = D // P                   # 32 feature chunks
NB = 512                      # matmul free dim for mm2 (psum bank limit)

# Asymmetric token groups: small first groups start production early.
GROUPS = [256, 256, 512]      # tokens per group (sum = TOK)
# per-group x chunking: ~1MB chunks; chunk of group with TB tokens
# holds KOC = (1MB / (TB*2*P)) ko-slices.
def _koc(tb):
    return (1 << 20) // (tb * 2 * P)

# Module-level knobs for test.py (harness never touches these)
TRACE = False
LAST_RESULTS = None

_nc_cache = None


def _build_program():
    from concourse import bacc, mybir, tile

    nc = bacc.Bacc(
        "TRN2", target_bir_lowering=False, debug=False, num_devices=NCORES
    )

    f32 = mybir.dt.float32
    bf16 = mybir.dt.bfloat16

    # one dram tensor per group (different chunk shapes)
    xts_dram = []
    for gi, tb in enumerate(GROUPS):
        koc = _koc(tb)
        nch = KO // koc
        xts_dram.append(
            nc.dram_tensor(f"xt{gi}", [nch, P, koc, tb], bf16, kind="ExternalInput")
        )
    at = nc.dram_tensor("at", [P, KO, R], bf16, kind="ExternalInput")
    bt = nc.dram_tensor("bt", [R, D], bf16, kind="ExternalInput")
    y = nc.dram_tensor("y", [TOK, D], bf16, kind="ExternalOutput")

    with tile.TileContext(nc) as tc:
        with (
            tc.tile_pool(name="consts", bufs=1) as consts,
            tc.tile_pool(name="xin", bufs=8) as xin,
            tc.tile_pool(name="tbuf", bufs=2) as tbuf,
            tc.tile_pool(name="yout", bufs=8) as yout,
            tc.tile_pool(name="pt", bufs=1, space="PSUM") as pt_pool,
            tc.tile_pool(name="py", bufs=6, space="PSUM") as py_pool,
            tc.tile_pool(name="warmp", bufs=1, space="PSUM") as warm_pool,
        ):
            # consts on the ACT ring; the SP ring head stays free for x.
            at_s = consts.tile([P, KO, R], bf16)
            nc.scalar.dma_start(at_s[:], at[:])
            bt_s = consts.tile([R, D], bf16)
            nc.scalar.dma_start(bt_s[:], bt[:])

            junk = consts.tile([P, NB], bf16)
            nc.gpsimd.memset(junk[:], 0.0)
            warm_t = warm_pool.tile([P, NB], f32)

            def pe_junk(n, free=256):
                # cheap PE filler: 16-wide stationary (LDWEIGHTS hides),
                # ~115ns each at K=8, 1/8 array energy.
                for _ in range(n):
                    nc.tensor.matmul(
                        warm_t[:R, :free], junk[:, :R], junk[:, :free],
                        start=True, stop=True, skip_group_check=True,
                    )

            for _ in range(8):
                pe_junk(1, free=NB)
            tc.no_sync_barrier()

            # Prefetch ALL x chunks on the SP ring, in token order,
            # before any y write can occupy the Sync queue.
            xtiles = {}
            for gi, tb in enumerate(GROUPS):
                koc = _koc(tb)
                for c in range(KO // koc):
                    t_ = xin.tile([P, koc, tb], bf16, tag="xt", name=f"x{gi}_{c}")
                    nc.sync.dma_start(t_[:], xts_dram[gi][c])
                    xtiles[(gi, c)] = t_

            def mm1_octet(gi, c, psum_t):
                tb = GROUPS[gi]
                koc = _koc(tb)
                for j in range(koc):
                    ko = c * koc + j
                    nc.tensor.matmul(
                        psum_t[:, :tb],
                        at_s[:, ko, :],
                        xtiles[(gi, c)][:, j, :],
                        start=(ko == 0),
                        stop=(ko == KO - 1),
                        skip_group_check=True,
                    )

            def make_tT(gi, psum_t):
                tb = GROUPS[gi]
                tT = tbuf.tile([R, tb], bf16, name=f"tT{gi}")
                nc.vector.tensor_copy(tT[:], psum_t[:, :tb])
                return tT

            def mm2_chunk(gi, c, tT, fill=0, split_write=1):
                row0 = sum(GROUPS[:gi]) + c * P
                y_row = yout.tile([P, D], bf16, name=f"yrow{gi}_{c}", tag="yrow")
                for n in range(D // NB):
                    psum_y = py_pool.tile([P, NB], f32, tag="psum_y")
                    nc.tensor.matmul(
                        psum_y[:],
                        tT[:, c * P : (c + 1) * P],
                        bt_s[:, n * NB : (n + 1) * NB],
                        start=True,
                        stop=True,
                        skip_group_check=True,
                    )
                    if fill:
                        pe_junk(fill)
                    # Single-bank PSUM evacuation, DVE : ACT alternating
                    if n % 2 == 0:
                        nc.vector.tensor_copy(y_row[:, n * NB : (n + 1) * NB], psum_y[:])
                    else:
                        nc.scalar.copy(y_row[:, n * NB : (n + 1) * NB], psum_y[:])
                # y writes issue from the Sync queue onto the SP ring.
                ds = D // split_write
                for s in range(split_write):
                    nc.sync.dma_start(
                        y[row0 : row0 + P, s * ds : (s + 1) * ds],
                        y_row[:, s * ds : (s + 1) * ds],
                    )

            # ---- pipelined schedule over asymmetric groups ----
            # group 0 octets (read-gated; pads fill the chunk gaps)
            NGR = len(GROUPS)
            prev = None  # (gi, tT) with mm2 pending
            for gi, tb in enumerate(GROUPS):
                psum_t = pt_pool.tile([R, NB], f32, tag="psum_t", name=f"pt{gi}")
                nch = KO // _koc(tb)
                pend = ([] if prev is None
                        else [(prev[0], c, prev[1]) for c in range(GROUPS[prev[0]] // P)])
                for c in range(nch):
                    mm1_octet(gi, c, psum_t)
                    if pend:
                        g2, c2, tT2 = pend.pop(0)
                        mm2_chunk(g2, c2, tT2)
                    else:
                        pe_junk(4)
                # any mm2 chunks that didn't fit between octets
                for g2, c2, tT2 in pend:
                    mm2_chunk(g2, c2, tT2)
                tT = make_tT(gi, psum_t)
                if gi == 0:
                    pe_junk(4)
                prev = (gi, tT)

            # final group's mm2: junk-filled, last chunk's write split.
            gi, tT = prev
            nyc = GROUPS[gi] // P
            for c in range(nyc):
                mm2_chunk(gi, c, tT, fill=1, split_write=(4 if c == nyc - 1 else 1))

    nc.finalize()
    return nc


def kernel(x, lora_matrix_B, lora_matrix_A):
    global _nc_cache, LAST_RESULTS
    import ml_dtypes
    from concourse.bass_utils import run_bass_kernel_spmd

    if _nc_cache is None:
        _nc_cache = _build_program()
    nc = _nc_cache

    bf16 = ml_dtypes.bfloat16
    x_flat = np.asarray(x, dtype=np.float32).reshape(NTOK, D).astype(bf16)
    A = np.asarray(lora_matrix_A, dtype=np.float32).astype(bf16)
    B = np.asarray(lora_matrix_B, dtype=np.float32).astype(bf16)

    # at[p, ko, j] = A[j, ko*128 + p];  bt[j, o] = B[o, j]
    at_prep = np.ascontiguousarray(A.reshape(R, KO, P).transpose(2, 1, 0))
    bt_prep = np.ascontiguousarray(B.T)

    in_maps = []
    for core in range(NCORES):
        xc = x_flat[core * TOK : (core + 1) * TOK, :]
        im = {"at": at_prep, "bt": bt_prep}
        t0 = 0
        for gi, tb in enumerate(GROUPS):
            koc = _koc(tb)
            nch = KO // koc
            xg = xc[t0 : t0 + tb, :]
            # xt[c, p, j, t] = xg[t, (c*koc + j)*128 + p]
            im[f"xt{gi}"] = np.ascontiguousarray(
                xg.reshape(tb, nch, koc, P).transpose(1, 3, 2, 0)
            )
            t0 += tb
        in_maps.append(im)

    res = run_bass_kernel_spmd(
        nc, in_maps, core_ids=list(range(NCORES)), trace=TRACE
    )
    LAST_RESULTS = res

    y = np.concatenate([res.results[c]["y"] for c in range(NCORES)], axis=0)
    return y.reshape(BATCH, SEQ, D).astype(np.float32)
